# revision 9
# baseline (speedup 1.0000x reference)
"""Sliding-window causal GQA attention block (QKV proj + RoPE + SDPA + out proj)
on 8 Trainium2 NeuronCores.

Sharding: 8 cores = 2 batches x 4 sequence chunks of 512 tokens. Each core
computes the full attention-block output for its (batch, seq-chunk):
  - Q projection for its 512 queries (all 16 heads) in transposed [d, s] layout
  - K/V projection for its chunk + 512-token halo (sliding window support)
  - RoPE via rotate-half permutation matmul + element-wise mul/add
  - attention tiled as (kv-head, 128-query tile) blocks: 4 GQA heads x 128
    queries = 512 matmul columns, 5 key tiles of 128 per query tile; scores
    kept transposed [keys, queries] so no on-chip transposes are needed
  - only the two diagonal key tiles carry an additive mask (applied on DVE
    before exp); the 3 interior tiles are mask-free.  Out-of-range keys for
    chunk 0 have K=V=0 (zero-padded x), contributing exp(0)=1 to the softmax
    denominator, which is corrected by a host-provided per-query-tile count.
  - softmax denominator: DVE tensor_reduce over the 5 exp tiles, then a
    single ones-vector matmul for the partition reduction
  - out-projection in 4 kv-group passes accumulated into SBUF fp32, so the
    tensor engine can overlap out-proj matmuls with later attention blocks.

Matmul operands are bf16 (x and weights pre-cast on the host); accumulation
is fp32 in PSUM; denominators/reciprocals fp32.
"""
import numpy as np

import concourse.bacc as bacc
import concourse.mybir as mybir
import concourse.tile as tile
from concourse.bass_utils import run_bass_kernel_spmd

# Problem constants (hardcoded per contract)
B, S, E = 2, 2048, 2048
H, KV, D = 16, 4, 128
WIN = 512
THETA = 1e6
NCORES = 8
CH = 512          # seq chunk per core
SW = 1024         # K/V window per core (halo 512 + own 512)
P = 128
ECH = E // P      # 16 contraction chunks
NJT = SW // P     # 8 key tiles in window
NQT = CH // P     # 4 query tiles per chunk
F32 = mybir.dt.float32
BF16 = mybir.dt.bfloat16
SCALE = 1.0 / float(np.sqrt(np.float32(D)))
MASK_NEG = -30000.0

_CACHE = {}


def _build():
    nc = bacc.Bacc("TRN2", target_bir_lowering=False, debug=False,
                   num_devices=NCORES)

    xt = nc.dram_tensor("xt", [E, SW], BF16, kind="ExternalInput")
    wqkv = nc.dram_tensor("wqkv", [E, (H + 2 * KV) * D], BF16, kind="ExternalInput")
    wo = nc.dram_tensor("wo", [H * D, E], BF16, kind="ExternalInput")
    cosw = nc.dram_tensor("cosw", [P, SW], F32, kind="ExternalInput")
    sinw = nc.dram_tensor("sinw", [P, SW], F32, kind="ExternalInput")
    # additive bias masks for the two diagonal key tiles of each query tile:
    # index mi = qt*2 + (0: r=0 tile, 1: r=4 tile); [128 keys, 512 cols]
    masks = nc.dram_tensor("masks", [2 * NQT, P, CH], BF16, kind="ExternalInput")
    # denominator correction: -#(implicitly-counted invalid interior keys)
    cden = nc.dram_tensor("cden", [1, NQT], F32, kind="ExternalInput")
    perm = nc.dram_tensor("perm", [P, P], BF16, kind="ExternalInput")
    ones = nc.dram_tensor("ones", [1, P], BF16, kind="ExternalInput")
    yt = nc.dram_tensor("yt", [E, CH], F32, kind="ExternalOutput")

    KOFF = H * D            # w_qkv column offsets
    VOFF = H * D + KV * D

    with tile.TileContext(nc) as tc:
        with (
            tc.tile_pool(name="res", bufs=1) as res,       # resident tensors
            tc.tile_pool(name="big", bufs=2) as big,       # x_halo/wv then o_T
            tc.tile_pool(name="wst", bufs=4) as wst,       # streamed w tiles
            tc.tile_pool(name="pep", bufs=2) as pep,       # exp score tiles
            tc.tile_pool(name="tmp", bufs=3) as tmp,       # transient compute
            tc.tile_pool(name="pj", bufs=3, space="PSUM") as pj,
            tc.tile_pool(name="ps1", bufs=2, space="PSUM") as ps1,  # scores
            tc.tile_pool(name="ps2", bufs=2, space="PSUM") as ps2,  # av
            tc.tile_pool(name="psd", bufs=1, space="PSUM") as psd,  # denom
        ):
            # ---------------- constants (gpsimd queue) ----------------------
            cos_sb = res.tile([P, SW], F32, tag="cosw")
            sin_sb = res.tile([P, SW], F32, tag="sinw")
            nc.gpsimd.dma_start(cos_sb[:], cosw.ap())
            nc.gpsimd.dma_start(sin_sb[:], sinw.ap())
            perm_sb = res.tile([P, P], BF16, tag="perm")
            nc.gpsimd.dma_start(perm_sb[:], perm.ap())
            ones_sb = res.tile([P, 1], BF16, tag="ones")
            nc.gpsimd.dma_start(ones_sb[:], ones.ap().rearrange("o p -> p o"))
            mask_sb = res.tile([P, 2 * NQT, CH], BF16, tag="masks")
            for mi in range(2 * NQT):
                nc.gpsimd.dma_start(mask_sb[:, mi, :], masks.ap()[mi])
            cden_sb = res.tile([1, NQT], F32, tag="cden")
            nc.gpsimd.dma_start(cden_sb[:], cden.ap())

            # ------------- x into SBUF (bf16, host pre-cast; HWDGE) ----------
            x_own = res.tile([P, ECH, CH], BF16, tag="xown")
            x_halo = big.tile([P, ECH, CH], BF16, tag="big")
            xt3 = xt.ap().rearrange("(eo p) s -> p eo s", p=P)
            wqkv3 = wqkv.ap().rearrange("(eo p) f -> p eo f", p=P)
            wk_t = {}

            def load_wk(fk):
                wk_t[fk] = wst.tile([P, ECH, P], BF16, tag="wqk",
                                    name=f"wk_{fk}")
                for eh in range(2):
                    sl = slice(eh * 8, eh * 8 + 8)
                    nc.sync.dma_start(
                        wk_t[fk][:, sl, :],
                        wqkv3[:, sl, KOFF + fk * P:KOFF + (fk + 1) * P])

            load_wk(0)
            for eh in range(4):
                sl = slice(eh * 4, eh * 4 + 4)
                nc.sync.dma_start(x_halo[:, sl, :], xt3[:, sl, 0:CH])
            for eh in range(4):
                sl = slice(eh * 4, eh * 4 + 4)
                nc.sync.dma_start(x_own[:, sl, :], xt3[:, sl, CH:SW])

            def x_win_slice(e, st):
                """lhsT [128 e-part, 128 s-cols] for window s-tile st (0..7)."""
                if st < 4:
                    return x_halo[:, e, st * P:(st + 1) * P]
                return x_own[:, e, (st - 4) * P:(st - 3) * P]

            # ---------------- rope helper ----------------
            def rope(dst, raw_ps, c0, c1, split4=False):
                """dst[128, n] = rope(raw) using cos/sin window cols [c0:c1)."""
                n = c1 - c0
                raw_sb = tmp.tile([P, CH], BF16, tag="qraw")
                nc.scalar.copy(out=raw_sb[:, :n], in_=raw_ps[:, :n])
                rot_ps = ps1.tile([P, CH], F32, tag="sc")
                nc.tensor.matmul(rot_ps[:, :n], perm_sb[:], raw_sb[:, :n],
                                 start=True, stop=True)
                t1 = tmp.tile([P, CH], F32, tag="qraw")
                nc.gpsimd.tensor_mul(out=t1[:, :n], in0=raw_sb[:, :n],
                                     in1=cos_sb[:, c0:c1])
                t2 = tmp.tile([P, CH], F32, tag="qraw")
                nc.vector.tensor_mul(out=t2[:, :n], in0=rot_ps[:, :n],
                                     in1=sin_sb[:, c0:c1])
                if split4:
                    nc.vector.tensor_add(
                        out=dst,
                        in0=t1[:, :n].rearrange("p (a b) -> p a b", a=NQT),
                        in1=t2[:, :n].rearrange("p (a b) -> p a b", a=NQT))
                else:
                    nc.vector.tensor_add(out=dst, in0=t1[:, :n], in1=t2[:, :n])

            # ------------- K projection (transposed [d, s] layout) -----------
            k_sb = res.tile([P, KV, SW], BF16, tag="k")
            for fk in range(KV):
                if fk > 0:
                    load_wk(fk)
                for sh in range(SW // CH):
                    k_ps = pj.tile([P, CH], F32, tag="pj")
                    for e in range(ECH):
                        nc.tensor.matmul(
                            k_ps[:], wk_t[fk][:, e, :],
                            (x_halo if sh == 0 else x_own)[:, e, :],
                            start=(e == 0), stop=(e == ECH - 1))
                    rope(k_sb[:, fk, sh * CH:(sh + 1) * CH], k_ps,
                         sh * CH, (sh + 1) * CH)

            # wv resident in one big-pool slot, [p, e_chunk, v_cols 512]
            wv_sb = big.tile([P, ECH, KV * D], BF16, tag="big")
            for eh in range(4):
                sl = slice(eh * 4, eh * 4 + 4)
                nc.gpsimd.dma_start(wv_sb[:, sl, :],
                                    wqkv3[:, sl, VOFF:VOFF + KV * D])

            # ------------- V projection (natural [s, d] layout) --------------
            v_sb = res.tile([P, NJT, KV * D], BF16, tag="v")
            for st in range(NJT):
                v_ps = pj.tile([P, KV * D], F32, tag="pj")
                for e in range(ECH):
                    nc.tensor.matmul(v_ps[:], x_win_slice(e, st), wv_sb[:, e, :],
                                     start=(e == 0), stop=(e == ECH - 1))
                nc.scalar.copy(out=v_sb[:, st, :], in_=v_ps[:])

            # ------------- Q proj + attention + out-proj, per kv group -------
            # q_sb free layout: block blk = kvb*4 + qt; cols = h4*128 + q
            q_sb = res.tile([P, H, CH], BF16, tag="q")
            o_sb = None
            wo3 = wo.ap().rearrange("(fo p) e2 -> p fo e2", p=P)

            def q_proj(kvb):
                for h4 in range(4):
                    fi = kvb * 4 + h4
                    wq_t = wst.tile([P, ECH, P], BF16, tag="wqk")
                    for eh in range(2):
                        sl = slice(eh * 8, eh * 8 + 8)
                        nc.sync.dma_start(wq_t[:, sl, :],
                                          wqkv3[:, sl, fi * P:(fi + 1) * P])
                    q_ps = pj.tile([P, CH], F32, tag="pj")
                    for e in range(ECH):
                        nc.tensor.matmul(q_ps[:], wq_t[:, e, :], x_own[:, e, :],
                                         start=(e == 0), stop=(e == ECH - 1))
                    dst = q_sb[:, kvb * 4:kvb * 4 + 4, h4 * P:(h4 + 1) * P]
                    rope(dst, q_ps, CH, SW, split4=True)

            def attn_block(kvb, qt):
                blk = kvb * 4 + qt
                pe_all = pep.tile([P, 5, CH], BF16, tag="pe",
                                  name=f"pe_{kvb}_{qt}")
                av_ps = ps2.tile([P, CH], F32, tag="av")
                dn_ps = psd.tile([1, CH], F32, tag="dn")
                for r in range(5):
                    w = qt + r
                    sc_ps = ps1.tile([P, CH], F32, tag="sc",
                                     name=f"sc_{kvb}_{qt}_{r}")
                    nc.tensor.matmul(sc_ps[:],
                                     k_sb[:, kvb, w * P:(w + 1) * P],
                                     q_sb[:, blk, :],
                                     start=True, stop=True)
                    if r == 0 or r == 4:
                        mi = qt * 2 + (0 if r == 0 else 1)
                        msk = tmp.tile([P, CH], BF16, tag="msk")
                        nc.vector.scalar_tensor_tensor(
                            out=msk[:], in0=sc_ps[:], scalar=SCALE,
                            in1=mask_sb[:, mi, :],
                            op0=mybir.AluOpType.mult,
                            op1=mybir.AluOpType.add)
                        nc.scalar.activation(
                            out=pe_all[:, r, :], in_=msk[:],
                            func=mybir.ActivationFunctionType.Exp)
                    else:
                        nc.scalar.activation(
                            out=pe_all[:, r, :], in_=sc_ps[:],
                            func=mybir.ActivationFunctionType.Exp,
                            scale=SCALE)
                    nc.tensor.matmul(av_ps[:],
                                     v_sb[:, w, kvb * D:(kvb + 1) * D],
                                     pe_all[:, r, :], start=(r == 0),
                                     stop=(r == 4))
                    nc.tensor.matmul(dn_ps[:], ones_sb[:], pe_all[:, r, :],
                                     start=(r == 0), stop=(r == 4))
                den = tmp.tile([1, CH], F32, tag="den")
                nc.vector.tensor_scalar_add(out=den[:], in0=dn_ps[:],
                                            scalar1=cden_sb[:, qt:qt + 1])
                bc = tmp.tile([P, CH], F32, tag="bc")
                nc.gpsimd.partition_broadcast(bc[:], den[:])
                rc = tmp.tile([P, CH], F32, tag="rc")
                nc.vector.reciprocal_approx_fast(out=rc[:], in_=bc[:])
                nc.vector.tensor_mul(out=o_sb[:, blk, :],
                                     in0=av_ps[:], in1=rc[:])

            q_proj(0)
            o_sb = big.tile([P, H, CH], BF16, tag="big")
            for qt in range(NQT):
                attn_block(0, qt)
            for kvb in range(1, KV):
                q_proj(kvb)
                for qt in range(NQT):
                    attn_block(kvb, qt)

            # ------------- out projection, transposed: yt = sum_f woT @ oT ---
            for et in range(ECH):
                wo_t = wst.tile([P, H, P], BF16, tag="wo")
                for fh in range(2):
                    sl = slice(fh * 8, fh * 8 + 8)
                    nc.sync.dma_start(
                        wo_t[:, sl, :], wo3[:, sl, et * P:(et + 1) * P])
                y_ps = pj.tile([P, CH], F32, tag="pj", name=f"y_{et}")
                for f in range(H):
                    kvb, h4 = f // 4, f % 4
                    o_f = o_sb[:, kvb * 4:kvb * 4 + 4, h4 * P:(h4 + 1) * P]
                    nc.tensor.matmul(y_ps[:], wo_t[:, f, :], o_f,
                                     start=(f == 0), stop=(f == H - 1))
                y_sb = tmp.tile([P, CH], F32, tag="ysb")
                nc.scalar.copy(out=y_sb[:], in_=y_ps[:])
                nc.sync.dma_start(yt.ap()[et * P:(et + 1) * P, :], y_sb[:])

    nc.compile()
    return nc


def _host_constants():
    inv_freq = (1.0 / (THETA ** (np.arange(0, D, 2, dtype=np.float32) / D))
                ).astype(np.float32)
    ang = np.arange(S, dtype=np.float32)[:, None] * inv_freq[None, :]
    emb = np.concatenate([ang, ang], axis=-1)          # [S, D]
    cos_t = np.ascontiguousarray(np.cos(emb).astype(np.float32).T)  # [D, S]
    sin_t = np.ascontiguousarray(np.sin(emb).astype(np.float32).T)

    import ml_dtypes
    pm = np.zeros((P, P), dtype=np.float32)            # rotate-half as lhsT
    a = np.arange(64)
    pm[a, a + 64] = 1.0
    pm[a + 64, a] = -1.0
    pm = pm.astype(ml_dtypes.bfloat16)

    onesv = np.ones((1, P), dtype=ml_dtypes.bfloat16)
    return cos_t, sin_t, pm, onesv


def _masks_for_chunk(chunk):
    """[8, 128, 512] bf16 additive masks for the two diagonal key tiles.

    mi = qt*2 + (0: window tile qt (r=0), 1: window tile qt+4 (r=4)).
    Columns are 4 heads x 128 queries; the mask depends only on the query
    position, so the four 128-col groups are equal."""
    import ml_dtypes
    m = np.full((2 * NQT, P, CH), MASK_NEG, dtype=np.float32)
    s0 = chunk * CH
    for qt in range(NQT):
        q_glob = s0 + qt * P + np.arange(P)[None, :]
        for ri, r in enumerate((0, 4)):
            w = qt + r
            jg0 = s0 - WIN + w * P
            j_glob = jg0 + np.arange(P)[:, None]
            dlt = q_glob - j_glob
            ok = (dlt >= 0) & (dlt < WIN) & (j_glob >= 0)
            tilem = np.where(ok, 0.0, MASK_NEG).astype(np.float32)
            m[qt * 2 + ri] = np.tile(tilem, (1, 4))
    return m.astype(ml_dtypes.bfloat16)


def _cden_for_chunk(chunk):
    """[1, 4] f32: minus the number of interior-tile keys with j_glob < 0.

    Interior key tiles (r = 1..3) are applied without masks; for chunk 0 the
    keys with j_glob < 0 have K=V=0 (zero-padded x), so each contributes
    exp(0)=1 to the denominator.  Count = clamp(3-qt, 0, 3)*128 for chunk 0."""
    c = np.zeros((1, NQT), dtype=np.float32)
    if chunk == 0:
        for qt in range(NQT):
            c[0, qt] = -float(max(0, 3 - qt) * P)
    return c


def _prepare_in_maps(x, w_qkv, w_o):
    import ml_dtypes
    cos_t, sin_t, pm, onesv = _host_constants()
    w_qkv = np.ascontiguousarray(w_qkv, dtype=np.float32).astype(ml_dtypes.bfloat16)
    w_o = np.ascontiguousarray(w_o, dtype=np.float32).astype(ml_dtypes.bfloat16)
    in_maps = []
    xts = [np.ascontiguousarray(np.asarray(x[b], dtype=np.float32).T
                                ).astype(ml_dtypes.bfloat16)
           for b in range(B)]
    for c in range(NCORES):
        b, chunk = divmod(c, 4)
        s0 = chunk * CH
        xt_win = np.zeros((E, SW), dtype=ml_dtypes.bfloat16)
        cos_win = np.zeros((P, SW), dtype=np.float32)
        sin_win = np.zeros((P, SW), dtype=np.float32)
        lo = s0 - WIN
        src_lo = max(0, lo)
        dst_lo = src_lo - lo
        xt_win[:, dst_lo:] = xts[b][:, src_lo:s0 + CH]
        cos_win[:, dst_lo:] = cos_t[:, src_lo:s0 + CH]
        sin_win[:, dst_lo:] = sin_t[:, src_lo:s0 + CH]
        in_maps.append({
            "xt": xt_win,
            "wqkv": w_qkv,
            "wo": w_o,
            "cosw": cos_win,
            "sinw": sin_win,
            "masks": _masks_for_chunk(chunk),
            "cden": _cden_for_chunk(chunk),
            "perm": pm,
            "ones": onesv,
        })
    return in_maps


def _install_ntff_shim():
    """bass_utils wants antenv.axon_hooks for trace=True under axon; this
    environment lacks that module, so synthesize it from the boot helper."""
    import sys
    import types
    if "antenv.axon_hooks" in sys.modules:
        return
    try:
        from trn_agent_boot.trn_boot import _ntff_profile_via_ctypes
        hook = _ntff_profile_via_ctypes("/opt/axon/libaxon_pjrt.so")
    except Exception:
        hook = None
    mod = types.ModuleType("antenv.axon_hooks")
    mod.get_axon_ntff_profile_hook = lambda: hook
    mod.set_axon_ntff_profile_hook = lambda h: None
    sys.modules["antenv.axon_hooks"] = mod


def run(x, w_qkv, w_o, trace=False):
    if "nc" not in _CACHE:
        _CACHE["nc"] = _build()
    nc = _CACHE["nc"]
    in_maps = _prepare_in_maps(np.asarray(x), np.asarray(w_qkv),
                               np.asarray(w_o))
    if trace:
        _install_ntff_shim()
    try:
        res = run_bass_kernel_spmd(nc, in_maps, list(range(NCORES)),
                                   trace=trace)
    except Exception:
        if not trace:
            raise
        res = run_bass_kernel_spmd(nc, in_maps, list(range(NCORES)),
                                   trace=False)
    y = np.empty((B, S, E), dtype=np.float32)
    for c in range(NCORES):
        b, chunk = divmod(c, 4)
        y[b, chunk * CH:(chunk + 1) * CH, :] = res.results[c]["yt"].T
    return y, res


def kernel(x, w_qkv, w_o):
    y, _ = run(x, w_qkv, w_o, trace=False)
    return y
